# revision 13
# baseline (speedup 1.0000x reference)
"""Dice coefficient metric kernel for TRN2 (8 NeuronCores, SPMD batch-parallel).

Reference computation (all fp32):
    inter[b,c] = sum_hw prd*tgt
    union[b,c] = sum_hw prd + sum_hw tgt + EPS
    dice[b,c]  = (2*inter + EPS) / union
    out[c]     = mean_b dice[b,c]

Sharding: batch dim (16) split across 8 cores -> 2 batches (8 (b,c) slabs
of 1024x1024) per core.  Slabs stream HBM->SBUF as [128, 4096] half-slab
f32 tiles (prd on the SP HWDGE ring, tgt on the ACT ring), 4-deep
buffering.  The 16 SDMA engines then run ~100% busy at ~26.5 GB/s each --
~97% of the 435 GB/s SBUF-AXI fabric ceiling -- so the ~158us stream is a
hard floor and everything else must hide behind or hug its edges.
(Measured dead ends: bf16 cast-during-DMA via SWDGE runs ~27% slower per
engine -- the M2S/read side still moves f32 and the cast path adds
overhead; descriptor size 4/8/16 KB leaves per-engine rate unchanged.)

Compute is split across engines so neither lags the stream: the DVE does
the inter reduction per unit (fused scalar_tensor_tensor mult+mult with
accum_out) and the ACT engine does the two plain sums (activation Copy
with accum_out); per half-tile pair that is DVE 4.4us + ACT 7.8us against
11.4us of DMA.  The last slab is split into three quarters + two eighths,
with the sums of two tail units moved to the DVE as fused union ops
(mult+add -> sum(pt+tt)) so each engine carries ~12us against the ~19us
tail DMA window and the post-stream drain is one eighth's compute.

Per-partition partials land in a zero-initialised stats tile
[A(inter|psum|tsum) | B(...)]; after the last accumulation a single
ones-vector matmul collapses the partition dim into PSUM, one DVE copy
moves the 60 floats to SBUF, and they are DMA'd out raw.  All remaining
folds and the dice arithmetic run on the host in fp64 while gathering the
8 per-core partials (fewer serialized device ops on the critical tail).
"""

import numpy as np

import concourse.bass as bass
import concourse.tile as tile
from concourse import bacc, mybir
from concourse.bass_utils import run_bass_kernel_spmd

B, C, H, W = 16, 4, 1024, 1024
N_CORES = 8
P = 128
EPS = 1e-6

B_LOC = B // N_CORES          # batches per core
SLABS = B_LOC * C             # (b,c) slabs per core
F = (H * W) // P              # free dim per full slab

N_FOLD = SLABS + 2            # columns per (group, kind)
N_OUT = 6 * N_FOLD            # raw stats columns DMA'd out per core


def _build_nc(slabs: int, feat: int, n_cores: int):
    """Build + compile the per-core Bass program (same program on all cores)."""
    nc = bacc.Bacc(
        "TRN2", target_bir_lowering=False, debug=False, num_devices=n_cores
    )
    f32 = mybir.dt.float32
    half = feat // 2
    quarter = feat // 4
    eighth = feat // 8
    n_fold = slabs + 2
    prd = nc.dram_tensor("prd", [slabs, P, feat], f32, kind="ExternalInput")
    tgt = nc.dram_tensor("tgt", [slabs, P, feat], f32, kind="ExternalInput")
    out = nc.dram_tensor("out", [1, 6 * n_fold], f32, kind="ExternalOutput")

    add = mybir.AluOpType.add
    mult = mybir.AluOpType.mult
    copy_f = mybir.ActivationFunctionType.Copy

    # (slab, col_offset, width, fold_group, fold_idx, sums_on) units.
    # Full slabs in halves with the sums on ACT; the last slab as three
    # quarters + two eighths, two of them with a fused DVE union instead
    # (sum(pt+tt) via scalar_tensor_tensor mult+add) to balance the tail.
    ls = slabs - 1
    units = []
    for s in range(ls):
        units.append((s, 0, half, 0, s, "act"))
        units.append((s, half, half, 1, s, "act"))
    # every third mid-stream unit fuses its union on the DVE to balance the
    # engines (DVE ~17.7us vs ACT ~15.6us per three tile pairs)
    units = [
        (s, off, w_, g, i, "dve" if k % 3 == 2 else so)
        for k, (s, off, w_, g, i, so) in enumerate(units)
    ]
    units.append((ls, 0, quarter, 0, ls, "act"))
    units.append((ls, quarter, quarter, 1, ls, "act"))
    units.append((ls, 2 * quarter, quarter, 0, ls + 1, "dve"))
    units.append((ls, 3 * quarter, eighth, 1, ls + 1, "act"))
    units.append((ls, 3 * quarter + eighth, eighth, 0, ls + 2, "dve"))

    # stats column layout: [A | B], each group = [inter | psum | tsum]
    # (DVE-union units put sum(pt)+sum(tt) in the psum column, tsum stays 0)
    K_INTER, K_PSUM, K_TSUM = 0, 1, 2

    def col(g, kind, i):
        return 3 * n_fold * g + kind * n_fold + i

    with tile.TileContext(nc) as tc:
        with (
            tc.tile_pool(name="io", bufs=4) as io_pool,
            tc.tile_pool(name="work", bufs=1) as work_pool,
            tc.tile_pool(name="psum", bufs=1, space=bass.MemorySpace.PSUM) as psum_pool,
        ):
            stats = work_pool.tile([P, 6 * n_fold], f32)
            nc.vector.memset(stats[:], 0.0)
            scr_v = work_pool.tile([P, half], f32)   # DVE main-out sink
            scr_a = work_pool.tile([P, half], f32)   # ACT main-out sink
            ones = work_pool.tile([P, 1], f32)
            nc.vector.memset(ones[:], 1.0)

            for s, off, width, g, i, sums_on in units:
                pt = io_pool.tile([P, width], f32, tag="prd")
                nc.sync.dma_start(pt[:], prd[s, :, off : off + width])
                tt = io_pool.tile([P, width], f32, tag="tgt")
                nc.scalar.dma_start(tt[:], tgt[s, :, off : off + width])

                # inter partial on the DVE: accum_out = sum((pt*1) * tt)
                nc.vector.scalar_tensor_tensor(
                    out=scr_v[:, 0:width], in0=pt[:], scalar=1.0, in1=tt[:],
                    op0=mult, op1=mult,
                    accum_out=stats[:, col(g, K_INTER, i) : col(g, K_INTER, i) + 1],
                )
                if sums_on == "dve":
                    # fused union on the DVE: accum_out = sum((pt*1) + tt)
                    nc.vector.scalar_tensor_tensor(
                        out=scr_v[:, 0:width], in0=pt[:], scalar=1.0, in1=tt[:],
                        op0=mult, op1=add,
                        accum_out=stats[:, col(g, K_PSUM, i) : col(g, K_PSUM, i) + 1],
                    )
                else:
                    # plain sums on the ACT engine (accumulating Copy)
                    nc.scalar.activation(
                        out=scr_a[:, 0:width], in_=pt[:], func=copy_f,
                        accum_out=stats[:, col(g, K_PSUM, i) : col(g, K_PSUM, i) + 1],
                    )
                    nc.scalar.activation(
                        out=scr_a[:, 0:width], in_=tt[:], func=copy_f,
                        accum_out=stats[:, col(g, K_TSUM, i) : col(g, K_TSUM, i) + 1],
                    )

            # Collapse the 128 partitions: ps[0, :] = ones.T @ stats (PSUM),
            # bounce to SBUF (DMA has no PSUM route) and ship raw.
            ps = psum_pool.tile([1, 6 * n_fold], f32)
            nc.tensor.matmul(ps[:], ones[:], stats[:], start=True, stop=True)
            raw = work_pool.tile([1, 6 * n_fold], f32)
            nc.vector.tensor_copy(raw[:], ps[:])
            nc.sync.dma_start(out[0:1, :], raw[:])

    nc.compile()
    return nc


def finalize(core_outs, slabs=SLABS, c=C, b=B):
    """Host-side fp64 reduction of the raw per-core stats vectors."""
    n_fold = slabs + 2
    total = np.zeros(c, dtype=np.float64)
    for o in core_outs:
        arr = np.asarray(o, dtype=np.float64).reshape(2, 3, n_fold)
        kinds = arr.sum(axis=0)                    # fold groups A+B
        # fold the tail columns (ls+1, ls+2) into the last-slab column
        per_slab = kinds[:, :slabs].copy()
        per_slab[:, slabs - 1] += kinds[:, slabs:].sum(axis=1)
        inter, psum, tsum = per_slab
        dice = (2.0 * inter + EPS) / (psum + tsum + EPS)   # (slabs,)
        total += dice.reshape(-1, c).sum(axis=0)           # fold local batches
    return (total / b).astype(np.float32)


_NC_CACHE: dict = {}


def _get_nc():
    key = (SLABS, F, N_CORES)
    if key not in _NC_CACHE:
        _NC_CACHE[key] = _build_nc(*key)
    return _NC_CACHE[key]


def _shard_inputs(prd: np.ndarray, tgt: np.ndarray):
    in_maps = []
    for i in range(N_CORES):
        sl = slice(i * B_LOC, (i + 1) * B_LOC)
        in_maps.append(
            {
                "prd": np.ascontiguousarray(prd[sl]).reshape(SLABS, P, F),
                "tgt": np.ascontiguousarray(tgt[sl]).reshape(SLABS, P, F),
            }
        )
    return in_maps


def kernel(prd: np.ndarray, tgt: np.ndarray, _trace: bool = False):
    prd = np.asarray(prd, dtype=np.float32)
    tgt = np.asarray(tgt, dtype=np.float32)
    assert prd.shape == (B, C, H, W) and tgt.shape == (B, C, H, W)

    nc = _get_nc()
    in_maps = _shard_inputs(prd, tgt)
    res = run_bass_kernel_spmd(nc, in_maps, list(range(N_CORES)), trace=_trace)
    out = finalize([r["out"] for r in res.results])
    if _trace:
        return out, res
    return out
